# revision 8
# baseline (speedup 1.0000x reference)
"""Trainium2 Bass kernel for nn_Attention_41704132444382.

Masked-linear QKV projection + 16-head attention + masked-linear output
projection. Sharding: batch x head-quad — core c handles batch c//4 and
heads (c%4)*4..(c%4)*4+3. Host sums the 4 per-batch partial outputs and
adds the gated bias.

Per-core layout (all matmul operands bf16; PSUM accumulates fp32):
  - x^T [1024 k, 2048 t] for its batch, on 8 k-partition tiles.
  - Q^T/K^T [64*2, 2048] per head-pair via weight-stationary matmuls.
  - V produced directly as [t, dv] tiles (x tile is the stationary
    operand) — no on-device transposes. A constant ones column at slot 64
    of each [128, 65] V tile makes the PV matmul emit the softmax
    denominator (colsum) on PSUM partition 64 for free.
  - Scores S^T [j keys, i queries]; exp on ScalarE straight from PSUM
    (scale 1/32 folded in; |scores/32| small so no max subtraction),
    one [128, 1024] activation per head-pair, double-buffered.
  - Normalization via PE transpose gather -> DVE reciprocal -> PE
    broadcast, fused into the attnT evacuation.
  - Out-projection accumulates the 4 heads in PSUM (K=128 chains over
    two head-pair attnT tiles); partial [2048, 1024] written fp32.

Emission is software-pipelined: block ib's QK projection chains are
emitted before block ib-1's normalization/out-projection so the PE fills
idle slots of the ScalarE-bound attention steady state.
"""

import sys

import numpy as np

sys.path.insert(0, "/opt/trn_rl_repo")

import concourse.bass as bass
import concourse.mybir as mybir
from concourse import bacc
from concourse.masks import make_identity
from concourse.tile import TileContext

DIM = 1024
HEADS = 16
B = 2
N = 2048  # tokens per batch = tokens per core
NCORES = 8
HPC = 4  # heads per core
DV = HPC * 64  # 256 head-dims per core
SCALE = DIM ** (-0.5)  # 1/32

F32 = mybir.dt.float32
BF16 = mybir.dt.bfloat16

NJT = N // 128  # 16 key tiles
NIB = 4  # query blocks of 512
IBW = 512  # i-block width


def build_nc():
    nc = bacc.Bacc("TRN2", target_bir_lowering=True)
    xT_d = nc.declare_dram_parameter("xT", [DIM, N], BF16, isOutput=False)
    wqkT_d = nc.declare_dram_parameter("wqkT", [DIM, 2 * DV], BF16, isOutput=False)
    wvT_d = nc.declare_dram_parameter("wvT", [DIM, DV], BF16, isOutput=False)
    woT_d = nc.declare_dram_parameter("woT", [DV, DIM], BF16, isOutput=False)
    out_d = nc.declare_dram_parameter("out", [N, DIM], F32, isOutput=True)

    mult = mybir.AluOpType.mult
    Exp = mybir.ActivationFunctionType.Exp

    with TileContext(nc) as tc:
        with tc.tile_pool(name="persist", bufs=1) as pp:
            wqk = pp.tile([128, 8 * 512], BF16)  # [k-part, (kt, qk-col)]
            wv = pp.tile([128, 8 * 256], BF16)  # [k-part, (kt, dv)]
            wo01 = pp.tile([128, 1024], BF16)  # [dv h0|h1, o]
            wo23 = pp.tile([128, 1024], BF16)
            xt = [pp.tile([128, N], BF16, name=f"xt{k}") for k in range(8)]
            qT = [pp.tile([128, N], BF16, name=f"qT{p}") for p in range(2)]  # pair p
            kT = [pp.tile([128, N], BF16, name=f"kT{p}") for p in range(2)]
            v_sb = pp.tile([128, NJT * HPC * 65], BF16)  # [t-part, (jt, h, dv|1)]
            ident = pp.tile([128, 128], F32)
            onesb = pp.tile([1, 64], BF16)

            # ---------- input DMAs ----------
            # weights on the scalar HWDGE ring (lands fast, not queued
            # behind x); x split into t-halves so QK chains for t-half 0
            # start after ~1MB instead of the full 4MB
            nc.scalar.dma_start(
                wqk[:].rearrange("p (kt o) -> p kt o", kt=8),
                wqkT_d[:].rearrange("(kt p) o -> p kt o", p=128),
            )
            nc.scalar.dma_start(
                wv[:].rearrange("p (kt o) -> p kt o", kt=8),
                wvT_d[:].rearrange("(kt p) o -> p kt o", p=128),
            )
            nc.scalar.dma_start(wo01[:], woT_d[0:128, :])
            nc.scalar.dma_start(wo23[:], woT_d[128:256, :])
            for th in range(4):
                for k in range(8):
                    eng = nc.sync if k % 2 == 0 else nc.gpsimd
                    eng.dma_start(
                        xt[k][:, th * 512 : (th + 1) * 512],
                        xT_d[k * 128 : (k + 1) * 128, th * 512 : (th + 1) * 512],
                    )

            make_identity(nc, ident[:])
            ones_f = pp.tile([128, 64], F32)
            nc.vector.memset(ones_f[:], 1.0)
            nc.vector.tensor_copy(onesb[:], ones_f[0:1, :])
            # ones column at slot 64 of each 65-wide V block (V writes 0..63)
            nc.vector.tensor_copy(
                v_sb[:].rearrange("p (b c) -> p b c", c=65)[:, :, 64:65],
                ones_f[:, 0 : NJT * HPC].rearrange("p (b c) -> p b c", c=1),
            )

            with (
                tc.tile_pool(name="spool", bufs=2, space="PSUM") as sp,
                tc.tile_pool(name="pvpool", bufs=2, space="PSUM") as pvp,
                tc.tile_pool(name="oppool", bufs=2, space="PSUM") as opp,
                tc.tile_pool(name="epool", bufs=3) as ep,
                tc.tile_pool(name="evac", bufs=2) as vp,
                tc.tile_pool(name="unpool", bufs=8) as up,
                tc.tile_pool(name="obpool", bufs=4) as obp,
            ):
                # ---------- K projection (all t), V direct, Q block 0 ----------
                def emit_qk(o, th):
                    # o: 0/1 -> q pair0/pair1, 2/3 -> k pair0/pair1
                    ps = opp.tile([128, 512], F32, tag="op", name=f"qk{o}_{th}")
                    for kt in range(8):
                        nc.tensor.matmul(
                            ps[:],
                            wqk[:, kt * 512 + o * 128 : kt * 512 + (o + 1) * 128],
                            xt[kt][:, th * 512 : (th + 1) * 512],
                            start=(kt == 0),
                            stop=(kt == 7),
                        )
                    dest = (qT + kT)[o]
                    nc.vector.tensor_copy(dest[:, th * 512 : (th + 1) * 512], ps[:])

                for th in range(4):
                    emit_qk(2, th)
                    emit_qk(3, th)
                for tt in range(16):  # V direct: [128 t, 256 dv]
                    ps = opp.tile([128, 512], F32, tag="op", name=f"vps{tt}")
                    for kt in range(8):
                        nc.tensor.matmul(
                            ps[:, 0:256],
                            xt[kt][:, tt * 128 : (tt + 1) * 128],
                            wv[:, kt * 256 : (kt + 1) * 256],
                            start=(kt == 0),
                            stop=(kt == 7),
                        )
                    nc.vector.tensor_copy(
                        v_sb[:, tt * 4 * 65 : (tt + 1) * 4 * 65].rearrange(
                            "p (h c) -> p h c", c=65
                        )[:, :, 0:64],
                        ps[:, 0:256].rearrange("p (h c) -> p h c", c=64),
                    )
                emit_qk(0, 0)
                emit_qk(1, 0)

                # ---------- attention blocks ----------
                prev = None  # deferred (norm + out-proj) state of prior block

                def emit_norm_outproj(st):
                    ib, cs_sb, unorm = st
                    # gather colsum chunks onto partitions: [1, 2048] -> [128, 16]
                    pt = opp.tile([128, 512], F32, tag="op", name=f"pt{ib}")
                    for i in range(16):  # i = h*4 + c
                        nc.tensor.transpose(
                            pt[:, i : i + 1],
                            cs_sb[0:1, i * 128 : (i + 1) * 128],
                            ident[0:1, 0:1],
                        )
                    rr = vp.tile([128, 16], F32, tag="rr", name=f"rr{ib}")
                    nc.vector.tensor_copy(rr[:], pt[:, 0:16])
                    rcp = vp.tile([128, 16], F32, tag="rcp", name=f"rcp{ib}")
                    nc.vector.reciprocal(rcp[:], rr[:])
                    # transpose reciprocal columns back to partition-0 rows
                    r2 = [
                        vp.tile([1, 512], BF16, tag=f"r2_{h}", name=f"r2_{ib}_{h}")
                        for h in range(4)
                    ]
                    for h in range(4):
                        pr = opp.tile([128, 512], F32, tag="op", name=f"pr{ib}_{h}")
                        for c in range(4):
                            nc.tensor.transpose(
                                pr[0:1, c * 128 : (c + 1) * 128],
                                rcp[:, h * 4 + c : h * 4 + c + 1],
                                ident[:],
                            )
                        nc.vector.tensor_copy(r2[h][0:1, :], pr[0:1, 0:512])
                    attnT = [
                        vp.tile([128, 512], BF16, tag=f"at{p}", name=f"at{ib}_{p}")
                        for p in range(2)
                    ]
                    for h in range(4):
                        rb = opp.tile([128, 512], F32, tag="op", name=f"rb{ib}_{h}")
                        nc.tensor.matmul(
                            rb[0:64, :], onesb[:], r2[h][:], start=True, stop=True
                        )
                        nc.vector.tensor_tensor(
                            attnT[h // 2][(h % 2) * 64 : (h % 2 + 1) * 64, :],
                            unorm[h][:],
                            rb[0:64, :],
                            mult,
                        )
                    # out-projection for this block
                    for tt in range(4):
                        for oh in range(2):
                            po = opp.tile(
                                [128, 512], F32, tag="op", name=f"po{ib}_{tt}_{oh}"
                            )
                            nc.tensor.matmul(
                                po[:],
                                attnT[0][:, tt * 128 : (tt + 1) * 128],
                                wo01[:, oh * 512 : (oh + 1) * 512],
                                start=True,
                                stop=False,
                            )
                            nc.tensor.matmul(
                                po[:],
                                attnT[1][:, tt * 128 : (tt + 1) * 128],
                                wo23[:, oh * 512 : (oh + 1) * 512],
                                start=False,
                                stop=True,
                            )
                            ob = obp.tile(
                                [128, 512], F32, tag="ob", name=f"ob{ib}_{tt}_{oh}"
                            )
                            nc.vector.tensor_copy(ob[:], po[:])
                            nc.sync.dma_start(
                                out_d[
                                    ib * 512 + tt * 128 : ib * 512 + (tt + 1) * 128,
                                    oh * 512 : (oh + 1) * 512,
                                ],
                                ob[:],
                            )

                for ib in range(NIB):
                    i0 = ib * IBW
                    cs_sb = vp.tile([1, 2048], F32, tag="cs", name=f"cs{ib}")
                    unorm = [
                        up.tile([64, 512], F32, tag="un", name=f"un{ib}_{h}")
                        for h in range(4)
                    ]
                    for pair in range(2):
                        pv = [
                            pvp.tile([65, 512], F32, tag="pv", name=f"pv{ib}_{pair}_{hh}")
                            for hh in range(2)
                        ]
                        for jt in range(NJT):
                            s = sp.tile([128, 1024], F32, tag="s", name=f"s{ib}_{pair}_{jt}")
                            # pre-load both head tiles into disjoint PE row
                            # groups so the two K=64 matmuls co-execute
                            for hh in range(2):
                                nc.tensor.ldweights(
                                    kT[pair][
                                        hh * 64 : (hh + 1) * 64,
                                        jt * 128 : (jt + 1) * 128,
                                    ],
                                    tile_position=(hh * 64, 0),
                                )
                            for hh in range(2):
                                nc.tensor.matmul(
                                    s[:, hh * 512 : (hh + 1) * 512],
                                    kT[pair][
                                        hh * 64 : (hh + 1) * 64,
                                        jt * 128 : (jt + 1) * 128,
                                    ],
                                    qT[pair][hh * 64 : (hh + 1) * 64, i0 : i0 + IBW],
                                    start=True,
                                    stop=True,
                                )
                            e = ep.tile([128, 1024], BF16, tag="e", name=f"e{ib}_{pair}_{jt}")
                            nc.scalar.activation(e[:], s[:], Exp, scale=SCALE)
                            for hh in range(2):
                                h = pair * 2 + hh
                                jv = (jt * 4 + h) * 65
                                nc.tensor.matmul(
                                    pv[hh][:],
                                    v_sb[:, jv : jv + 65],
                                    e[:, hh * 512 : (hh + 1) * 512],
                                    start=(jt == 0),
                                    stop=(jt == NJT - 1),
                                )
                        for hh in range(2):
                            h = pair * 2 + hh
                            nc.vector.tensor_copy(
                                cs_sb[0:1, h * 512 : (h + 1) * 512], pv[hh][64:65, :]
                            )
                            nc.vector.tensor_copy(unorm[h][:], pv[hh][0:64, :])
                    # prefetch next block's Q projection before this block's
                    # norm/out-proj so its PE work schedules early
                    if ib + 1 < NIB:
                        emit_qk(0, ib + 1)
                        emit_qk(1, ib + 1)
                    if prev is not None:
                        emit_norm_outproj(prev)
                    prev = (ib, cs_sb, unorm)

                emit_norm_outproj(prev)

    nc.compile()
    return nc


_NC = None


def _get_nc():
    global _NC
    if _NC is None:
        _NC = build_nc()
    return _NC


def _gate(mask):
    """Exact jax fp32 gate: sigmoid(m) > 0.5 (fp32 logistic rounding)."""
    mask = np.asarray(mask, dtype=np.float32)
    return (np.float32(1.0) / (np.float32(1.0) + np.exp(-mask))) > np.float32(0.5)


def make_in_maps(x, qkv_weight, qkv_weight_mask, out_weight, out_weight_mask):
    import ml_dtypes

    bf = ml_dtypes.bfloat16
    x = np.asarray(x, dtype=np.float32)
    wq = np.asarray(qkv_weight, dtype=np.float32) * _gate(qkv_weight_mask)
    wo = np.asarray(out_weight, dtype=np.float32) * _gate(out_weight_mask)

    in_maps = []
    for c in range(NCORES):
        b, g = divmod(c, 4)
        r0 = g * DV
        xT = np.ascontiguousarray(x[b].T.astype(bf))
        wqk = np.concatenate(
            [wq[r0 : r0 + DV], wq[DIM + r0 : DIM + r0 + DV]], axis=0
        )  # [512, 1024] rows = (q h0..h3 | k h0..h3)
        in_maps.append(
            {
                "xT": xT,
                "wqkT": np.ascontiguousarray(wqk.T.astype(bf)),
                "wvT": np.ascontiguousarray(wq[2 * DIM + r0 : 2 * DIM + r0 + DV].T.astype(bf)),
                "woT": np.ascontiguousarray(wo[:, r0 : r0 + DV].T.astype(bf)),
            }
        )
    return in_maps


LAST_RESULTS = None  # BassKernelResults of the most recent run (for profiling)


def kernel(
    x,
    qkv_weight,
    qkv_weight_mask,
    out_weight,
    out_weight_mask,
    out_bias,
    out_bias_mask,
    _trace=False,
    _tmpdir=None,
):
    global LAST_RESULTS
    from concourse.bass_utils import run_bass_kernel_spmd

    nc = _get_nc()
    in_maps = make_in_maps(x, qkv_weight, qkv_weight_mask, out_weight, out_weight_mask)
    res = run_bass_kernel_spmd(
        nc, in_maps, list(range(NCORES)), trace=_trace, tmpdir=_tmpdir
    )
    LAST_RESULTS = res
    out = np.zeros((B, N, DIM), dtype=np.float32)
    for c, r in enumerate(res.results):
        out[c // 4] += r["out"]
    out_bias = np.asarray(out_bias, dtype=np.float32)
    out += np.where(_gate(out_bias_mask), out_bias, np.float32(0.0))[None, None, :]
    return out


# revision 10
# speedup vs baseline: 1.1350x; 1.1350x over previous
"""Trainium2 Bass kernel for nn_Attention_41704132444382.

Masked-linear QKV projection + 16-head attention + masked-linear output
projection. Sharding: batch x head-quad — core c handles batch c//4 and
heads (c%4)*4..(c%4)*4+3. Host sums the 4 per-batch partial outputs and
adds the gated bias.

Per-core layout (all matmul operands bf16; PSUM accumulates fp32):
  - x^T [1024 k, 2048 t] for its batch, on 8 k-partition tiles.
  - Q^T/K^T [64*2, 2048] per head-pair via weight-stationary matmuls.
  - V produced directly as [t, dv] tiles (x tile is the stationary
    operand) — no on-device transposes. A constant ones column at slot 64
    of each [128, 65] V tile makes the PV matmul emit the softmax
    denominator (colsum) on PSUM partition 64 for free.
  - Scores S^T [j keys, i queries]; exp on ScalarE straight from PSUM
    (scale 1/32 folded in; |scores/32| small so no max subtraction),
    one [128, 1024] activation per head-pair, double-buffered.
  - Normalization via PE transpose gather -> DVE reciprocal -> PE
    broadcast, fused into the attnT evacuation.
  - Out-projection accumulates the 4 heads in PSUM (K=128 chains over
    two head-pair attnT tiles); partial [2048, 1024] written fp32.

Emission is software-pipelined: block ib's QK projection chains are
emitted before block ib-1's normalization/out-projection so the PE fills
idle slots of the ScalarE-bound attention steady state.
"""

import sys

import numpy as np

sys.path.insert(0, "/opt/trn_rl_repo")

import concourse.bass as bass
import concourse.mybir as mybir
from concourse import bacc
from concourse.masks import make_identity
from concourse.tile import TileContext

DIM = 1024
HEADS = 16
B = 2
N = 2048  # tokens per batch = tokens per core
NCORES = 8
HPC = 4  # heads per core
DV = HPC * 64  # 256 head-dims per core
SCALE = DIM ** (-0.5)  # 1/32

F32 = mybir.dt.float32
F32R = mybir.dt.float32r
BF16 = mybir.dt.bfloat16

NJT = N // 128  # 16 key tiles
NIB = 4  # query blocks of 512
IBW = 512  # i-block width


def build_nc():
    nc = bacc.Bacc("TRN2", target_bir_lowering=True)
    xT_d = nc.declare_dram_parameter("xT", [DIM, N], F32R, isOutput=False)
    wqkT_d = nc.declare_dram_parameter("wqkT", [DIM, 2 * DV], F32R, isOutput=False)
    wvT_d = nc.declare_dram_parameter("wvT", [DIM, DV], F32R, isOutput=False)
    woT_d = nc.declare_dram_parameter("woT", [DV, DIM], BF16, isOutput=False)
    out_d = nc.declare_dram_parameter("out", [N, DIM], F32, isOutput=True)

    mult = mybir.AluOpType.mult
    Exp = mybir.ActivationFunctionType.Exp

    with TileContext(nc) as tc:
        with tc.tile_pool(name="persist", bufs=1) as pp:
            wqk = pp.tile([128, 8 * 512], F32R)  # [k-part, (kt, qk-col)]
            wv = pp.tile([128, 8 * 256], F32R)  # [k-part, (kt, dv)]
            wo01 = pp.tile([128, 1024], BF16)  # [dv h0|h1, o]
            wo23 = pp.tile([128, 1024], BF16)
            xt = [pp.tile([128, N], F32R, name=f"xt{k}") for k in range(8)]
            qT = [pp.tile([128, N], BF16, name=f"qT{p}") for p in range(2)]  # pair p
            kT = [pp.tile([128, N], BF16, name=f"kT{p}") for p in range(2)]
            v_sb = pp.tile([128, NJT * HPC * 65], BF16)  # [t-part, (jt, h, dv|1)]
            ident = pp.tile([128, 128], F32)
            onesb = pp.tile([1, 64], BF16)

            # ---------- input DMAs ----------
            # weights on the scalar HWDGE ring (lands fast, not queued
            # behind x); x split into t-halves so QK chains for t-half 0
            # start after ~1MB instead of the full 4MB
            nc.scalar.dma_start(
                wqk[:].rearrange("p (kt o) -> p kt o", kt=8),
                wqkT_d[:].rearrange("(kt p) o -> p kt o", p=128),
            )
            nc.scalar.dma_start(
                wv[:].rearrange("p (kt o) -> p kt o", kt=8),
                wvT_d[:].rearrange("(kt p) o -> p kt o", p=128),
            )
            nc.scalar.dma_start(wo01[:], woT_d[0:128, :])
            nc.scalar.dma_start(wo23[:], woT_d[128:256, :])
            for th in range(4):
                for k in range(8):
                    nc.sync.dma_start(
                        xt[k][:, th * 512 : (th + 1) * 512],
                        xT_d[k * 128 : (k + 1) * 128, th * 512 : (th + 1) * 512],
                    )

            make_identity(nc, ident[:])
            ones_f = pp.tile([128, 64], F32)
            nc.vector.memset(ones_f[:], 1.0)
            nc.vector.tensor_copy(onesb[:], ones_f[0:1, :])
            # ones column at slot 64 of each 65-wide V block (V writes 0..63)
            nc.vector.tensor_copy(
                v_sb[:].rearrange("p (b c) -> p b c", c=65)[:, :, 64:65],
                ones_f[:, 0 : NJT * HPC].rearrange("p (b c) -> p b c", c=1),
            )

            with (
                tc.tile_pool(name="spool", bufs=2, space="PSUM") as sp,
                tc.tile_pool(name="pvpool", bufs=2, space="PSUM") as pvp,
                tc.tile_pool(name="oppool", bufs=2, space="PSUM") as opp,
                tc.tile_pool(name="epool", bufs=3) as ep,
                tc.tile_pool(name="evac", bufs=2) as vp,
                tc.tile_pool(name="unpool", bufs=8) as up,
                tc.tile_pool(name="obpool", bufs=4) as obp,
            ):
                # ---------- K projection (all t), V direct, Q block 0 ----------
                def emit_qk(o, th):
                    # o: 0/1 -> q pair0/pair1, 2/3 -> k pair0/pair1
                    ps = opp.tile([128, 512], F32, tag="op", name=f"qk{o}_{th}")
                    for kt in range(8):
                        nc.tensor.matmul(
                            ps[:],
                            wqk[:, kt * 512 + o * 128 : kt * 512 + (o + 1) * 128],
                            xt[kt][:, th * 512 : (th + 1) * 512],
                            start=(kt == 0),
                            stop=(kt == 7),
                        )
                    dest = (qT + kT)[o]
                    nc.vector.tensor_copy(dest[:, th * 512 : (th + 1) * 512], ps[:])

                for th in range(4):
                    emit_qk(2, th)
                    emit_qk(3, th)
                for tt in range(16):  # V direct: [128 t, 256 dv]
                    ps = opp.tile([128, 512], F32, tag="op", name=f"vps{tt}")
                    for kt in range(8):
                        nc.tensor.matmul(
                            ps[:, 0:256],
                            xt[kt][:, tt * 128 : (tt + 1) * 128],
                            wv[:, kt * 256 : (kt + 1) * 256],
                            start=(kt == 0),
                            stop=(kt == 7),
                        )
                    nc.vector.tensor_copy(
                        v_sb[:, tt * 4 * 65 : (tt + 1) * 4 * 65].rearrange(
                            "p (h c) -> p h c", c=65
                        )[:, :, 0:64],
                        ps[:, 0:256].rearrange("p (h c) -> p h c", c=64),
                    )
                emit_qk(0, 0)
                emit_qk(1, 0)

                # ---------- attention blocks ----------
                prev = None  # deferred (norm + out-proj) state of prior block

                def emit_norm_outproj(st):
                    ib, cs_sb, unorm = st
                    # gather colsum chunks onto partitions: [1, 2048] -> [128, 16]
                    pt = opp.tile([128, 512], F32, tag="op", name=f"pt{ib}")
                    for i in range(16):  # i = h*4 + c
                        nc.tensor.transpose(
                            pt[:, i : i + 1],
                            cs_sb[0:1, i * 128 : (i + 1) * 128],
                            ident[0:1, 0:1],
                        )
                    rr = vp.tile([128, 16], F32, tag="rr", name=f"rr{ib}")
                    nc.vector.tensor_copy(rr[:], pt[:, 0:16])
                    rcp = vp.tile([128, 16], F32, tag="rcp", name=f"rcp{ib}")
                    nc.vector.reciprocal(rcp[:], rr[:])
                    # transpose reciprocal columns back to partition-0 rows
                    r2 = [
                        vp.tile([1, 512], BF16, tag=f"r2_{h}", name=f"r2_{ib}_{h}")
                        for h in range(4)
                    ]
                    for h in range(4):
                        pr = opp.tile([128, 512], F32, tag="op", name=f"pr{ib}_{h}")
                        for c in range(4):
                            nc.tensor.transpose(
                                pr[0:1, c * 128 : (c + 1) * 128],
                                rcp[:, h * 4 + c : h * 4 + c + 1],
                                ident[:],
                            )
                        nc.vector.tensor_copy(r2[h][0:1, :], pr[0:1, 0:512])
                    attnT = [
                        vp.tile([128, 512], BF16, tag=f"at{p}", name=f"at{ib}_{p}")
                        for p in range(2)
                    ]
                    for h in range(4):
                        rb = opp.tile([128, 512], F32, tag="op", name=f"rb{ib}_{h}")
                        nc.tensor.matmul(
                            rb[0:64, :], onesb[:], r2[h][:], start=True, stop=True
                        )
                        nc.vector.tensor_tensor(
                            attnT[h // 2][(h % 2) * 64 : (h % 2 + 1) * 64, :],
                            unorm[h][:],
                            rb[0:64, :],
                            mult,
                        )
                    # out-projection for this block
                    for tt in range(4):
                        for oh in range(2):
                            po = opp.tile(
                                [128, 512], F32, tag="op", name=f"po{ib}_{tt}_{oh}"
                            )
                            nc.tensor.matmul(
                                po[:],
                                attnT[0][:, tt * 128 : (tt + 1) * 128],
                                wo01[:, oh * 512 : (oh + 1) * 512],
                                start=True,
                                stop=False,
                            )
                            nc.tensor.matmul(
                                po[:],
                                attnT[1][:, tt * 128 : (tt + 1) * 128],
                                wo23[:, oh * 512 : (oh + 1) * 512],
                                start=False,
                                stop=True,
                            )
                            ob = obp.tile(
                                [128, 512], F32, tag="ob", name=f"ob{ib}_{tt}_{oh}"
                            )
                            nc.vector.tensor_copy(ob[:], po[:])
                            nc.sync.dma_start(
                                out_d[
                                    ib * 512 + tt * 128 : ib * 512 + (tt + 1) * 128,
                                    oh * 512 : (oh + 1) * 512,
                                ],
                                ob[:],
                            )

                for ib in range(NIB):
                    i0 = ib * IBW
                    cs_sb = vp.tile([1, 2048], F32, tag="cs", name=f"cs{ib}")
                    unorm = [
                        up.tile([64, 512], F32, tag="un", name=f"un{ib}_{h}")
                        for h in range(4)
                    ]
                    for pair in range(2):
                        pv = [
                            pvp.tile([65, 512], F32, tag="pv", name=f"pv{ib}_{pair}_{hh}")
                            for hh in range(2)
                        ]
                        for jt in range(NJT):
                            s = sp.tile([128, 1024], F32, tag="s", name=f"s{ib}_{pair}_{jt}")
                            for hh in range(2):
                                nc.tensor.matmul(
                                    s[:, hh * 512 : (hh + 1) * 512],
                                    kT[pair][
                                        hh * 64 : (hh + 1) * 64,
                                        jt * 128 : (jt + 1) * 128,
                                    ],
                                    qT[pair][hh * 64 : (hh + 1) * 64, i0 : i0 + IBW],
                                    start=True,
                                    stop=True,
                                )
                            e = ep.tile([128, 1024], BF16, tag="e", name=f"e{ib}_{pair}_{jt}")
                            nc.scalar.activation(e[:], s[:], Exp, scale=SCALE)
                            for hh in range(2):
                                h = pair * 2 + hh
                                jv = (jt * 4 + h) * 65
                                nc.tensor.matmul(
                                    pv[hh][:],
                                    v_sb[:, jv : jv + 65],
                                    e[:, hh * 512 : (hh + 1) * 512],
                                    start=(jt == 0),
                                    stop=(jt == NJT - 1),
                                )
                        for hh in range(2):
                            h = pair * 2 + hh
                            nc.vector.tensor_copy(
                                cs_sb[0:1, h * 512 : (h + 1) * 512], pv[hh][64:65, :]
                            )
                            nc.vector.tensor_copy(unorm[h][:], pv[hh][0:64, :])
                    # prefetch next block's Q projection before this block's
                    # norm/out-proj so its PE work schedules early
                    if ib + 1 < NIB:
                        emit_qk(0, ib + 1)
                        emit_qk(1, ib + 1)
                    if prev is not None:
                        emit_norm_outproj(prev)
                    prev = (ib, cs_sb, unorm)

                emit_norm_outproj(prev)

    nc.compile()
    return nc


_NC = None


def _get_nc():
    global _NC
    if _NC is None:
        _NC = build_nc()
    return _NC


def _gate(mask):
    """Exact jax fp32 gate: sigmoid(m) > 0.5 (fp32 logistic rounding)."""
    mask = np.asarray(mask, dtype=np.float32)
    return (np.float32(1.0) / (np.float32(1.0) + np.exp(-mask))) > np.float32(0.5)


def make_in_maps(x, qkv_weight, qkv_weight_mask, out_weight, out_weight_mask):
    import ml_dtypes

    bf = ml_dtypes.bfloat16
    x = np.asarray(x, dtype=np.float32)
    wq = np.asarray(qkv_weight, dtype=np.float32) * _gate(qkv_weight_mask)
    wo = np.asarray(out_weight, dtype=np.float32) * _gate(out_weight_mask)

    in_maps = []
    for c in range(NCORES):
        b, g = divmod(c, 4)
        r0 = g * DV
        xT = np.ascontiguousarray(x[b].T)
        wqk = np.concatenate(
            [wq[r0 : r0 + DV], wq[DIM + r0 : DIM + r0 + DV]], axis=0
        )  # [512, 1024] rows = (q h0..h3 | k h0..h3)
        in_maps.append(
            {
                "xT": xT,
                "wqkT": np.ascontiguousarray(wqk.T),
                "wvT": np.ascontiguousarray(wq[2 * DIM + r0 : 2 * DIM + r0 + DV].T),
                "woT": np.ascontiguousarray(wo[:, r0 : r0 + DV].T.astype(bf)),
            }
        )
    return in_maps


LAST_RESULTS = None  # BassKernelResults of the most recent run (for profiling)


def kernel(
    x,
    qkv_weight,
    qkv_weight_mask,
    out_weight,
    out_weight_mask,
    out_bias,
    out_bias_mask,
    _trace=False,
    _tmpdir=None,
):
    global LAST_RESULTS
    from concourse.bass_utils import run_bass_kernel_spmd

    nc = _get_nc()
    in_maps = make_in_maps(x, qkv_weight, qkv_weight_mask, out_weight, out_weight_mask)
    res = run_bass_kernel_spmd(
        nc, in_maps, list(range(NCORES)), trace=_trace, tmpdir=_tmpdir
    )
    LAST_RESULTS = res
    out = np.zeros((B, N, DIM), dtype=np.float32)
    for c, r in enumerate(res.results):
        out[c // 4] += r["out"]
    out_bias = np.asarray(out_bias, dtype=np.float32)
    out += np.where(_gate(out_bias_mask), out_bias, np.float32(0.0))[None, None, :]
    return out
